# revision 16
# baseline (speedup 1.0000x reference)
"""Trainium2 Bass kernel for nn_Encoding3D (vq_codebook), v2.

Math per voxel feature x = X[b,d,n] (N = T*H*W):
    logit_k = scale[k,d]*(x-cw[k,d])^2 = a*u + b*v + c   (u=x^2, v=x,
              a=s, b=-2sc, c=sc^2)
    A = softmax_k(logit);  E[b,n,d] = x - (sum_k e_k cw_k)/(sum_k e_k)
    gamma = sigmoid((sum_n E) @ fc_w.T / K + fc_b);  out = relu(E*(1+gamma))

Sharding: 8 cores = (b in 0..3) x (N-half in 0..1); one pairwise 256B
AllReduce of sum_n E, triggered ASAP after the last chunk's ratio.

Per-core pipeline (4096 voxels, 4 chunks of 1024, 16 groups of 4 ch):
  basis bt[128, CH] fp16: rows 0:64 = v = fp16(x), rows 64:128 = u = x^2
        (x is DMA'd into both halves of xs so DVE ops stay same-partition)
  PE:   logits[(4ch,32k)=128, n] = coefT_g.T @ bt   (contraction 128)
  exp:  13/16 groups on ACT: e = Exp(logits + cbias_g) -> fp16
        3/16 groups on DVE: fast-exp e = bitcast16(satu16(A16*logits+cb2))
        (2^x bit trick; ~3% e error -> ~1e-3 on E since |cw| <= 0.022)
  PE:   sums[128, n] += selT_g.T @ e  (s0 rows 0:64, s1 rows 64:128)
  DVE:  r=1/s0; ncorr=-(s1*r) with accum -> egp; E = x + ncorr
  tail: S = sum(egp cols: -corr sums + x sums) -> AllReduce(pairs) ->
        gamma -> out = relu(E*(1+gamma)) split DVE/ACT, DMA on 4 queues.
"""

import numpy as np

import concourse.bacc as bacc
import concourse.bass as bass
import concourse.mybir as mybir
import concourse.tile as tile
from concourse.bass_utils import run_bass_kernel_spmd

B, D, K = 4, 64, 32
T, H, W = 8, 32, 32
N = T * H * W            # 8192
NCORES = 8
NL = N // 2              # 4096 voxels per core
CH = 1024                # chunk (free-dim) size
NCH = NL // CH           # 4 chunks
NG = D // 4              # 16 groups of 4 channels
f32 = mybir.dt.float32
f16 = mybir.dt.float16
u16 = mybir.dt.uint16

AF = mybir.ActivationFunctionType
ALU = mybir.AluOpType

A16 = 1024.0 / np.log(2.0)       # fast-exp scale for fp16 bitcast
B16 = 15.0 * 1024.0 - 44.0       # fp16 exponent bias + sawtooth centering
DVEG = (4, 6, 9, 12, 14)         # groups whose exp runs on DVE (fast-exp)
NWARM = 16


def _build_nc(use_collective=True):
    nc = bacc.Bacc("TRN2", target_bir_lowering=False, debug=False,
                   num_devices=NCORES if use_collective else 1)

    x_d = nc.dram_tensor("x", [D, NL], f32, kind="ExternalInput")
    coefT_d = nc.dram_tensor("coefT", [128, 128 * NG], f16, kind="ExternalInput")
    selT_d = nc.dram_tensor("selT", [128, 64 + 128 * NG], f16, kind="ExternalInput")
    cbx_d = nc.dram_tensor("cbx", [128, 2 * NG], f32, kind="ExternalInput")
    fcwT_d = nc.dram_tensor("fcwT", [D, D], f16, kind="ExternalInput")
    nfcb_d = nc.dram_tensor("nfcb", [D, 1], f32, kind="ExternalInput")
    out_d = nc.dram_tensor("out", [D, NL], f32, kind="ExternalOutput")

    with tile.TileContext(nc) as tc:
        with (
            tc.tile_pool(name="const", bufs=1) as cpool,
            tc.tile_pool(name="basis", bufs=2) as bpool,
            tc.tile_pool(name="ework", bufs=4) as epool,
            tc.tile_pool(name="fin", bufs=2) as finpool,
            tc.tile_pool(name="persist", bufs=1) as ppool,
            tc.tile_pool(name="psumL", bufs=3, space=bass.MemorySpace.PSUM) as psL,
            tc.tile_pool(name="psumS", bufs=1, space=bass.MemorySpace.PSUM) as psS,
            tc.tile_pool(name="dram", bufs=1, space="DRAM") as dpool,
        ):
            coefT = cpool.tile([128, 128 * NG], f16, tag="coefT")
            selT = cpool.tile([128, 64 + 128 * NG], f16, tag="selT")
            cbx = cpool.tile([128, 2 * NG], f32, tag="cbx")
            fcwT = cpool.tile([D, D], f16, tag="fcwT")
            nfcb = cpool.tile([D, 1], f32, tag="nfcb")
            wrm = cpool.tile([128, 512], f16, tag="wrm")
            xs = ppool.tile([128, NL], f32, tag="xs")   # x in both halves
            Et = ppool.tile([D, NL], f32, tag="Et")
            egp = ppool.tile([D, 8], f32, tag="egp")    # 0:4 -corr, 4:8 x

            # ---- input DMAs: chunk 0 first in 512-col halves, split over
            # sync/scalar/gpsimd queues so the pipeline can start ~2.5us in.
            HC = 512
            nc.vector.memset(wrm[:], 0.5)
            nc.sync.dma_start(xs[0:D, 0:HC], x_d[:, 0:HC])
            nc.sync.dma_start(xs[D:128, 0:HC], x_d[:, 0:HC])
            nc.gpsimd.dma_start(xs[0:D, HC:CH], x_d[:, HC:CH])
            nc.gpsimd.dma_start(xs[D:128, HC:CH], x_d[:, HC:CH])
            nc.scalar.dma_start(cbx[:], cbx_d[:])
            nc.scalar.dma_start(coefT[:, 0:256], coefT_d[:, 0:256])
            nc.sync.dma_start(selT[:, 64:320], selT_d[:, 64:320])
            nc.scalar.dma_start(coefT[:, 256:2048], coefT_d[:, 256:2048])
            nc.scalar.dma_start(selT[:, 320:2112], selT_d[:, 320:2112])
            for cc_ in range(1, NCH):
                cs = slice(cc_ * CH, (cc_ + 1) * CH)
                nc.sync.dma_start(xs[0:D, cs], x_d[:, cs])
                nc.gpsimd.dma_start(xs[D:128, cs], x_d[:, cs])
            nc.gpsimd.dma_start(fcwT[:], fcwT_d[:])
            nc.gpsimd.dma_start(nfcb[:], nfcb_d[:])

            # PE warm-up: start the HAM busy window before real work is
            # ready (cold PE runs matmuls at 1.2 GHz for the first ~3.4us
            # of activity; keep the queue from idling after that).
            warm = psS.tile([128, 512], f32, tag="sums", name="warm")
            for _ in range(NWARM):
                nc.tensor.matmul(warm[:], wrm[:, 0:128], wrm[:],
                                 start=True, stop=True)

            basis = {}

            def basis_prep(c, halves=False):
                c0 = c * CH
                bt = bpool.tile([128, CH], f16, tag="bt")
                basis[c] = bt
                pieces = ((0, HC), (HC, CH)) if halves else ((0, CH),)
                for lo, hi in pieces:
                    nc.vector.tensor_copy(bt[0:D, lo:hi],
                                          xs[0:D, c0 + lo:c0 + hi])
                    nc.vector.tensor_tensor(bt[D:128, lo:hi],
                                            xs[D:128, c0 + lo:c0 + hi],
                                            xs[D:128, c0 + lo:c0 + hi],
                                            ALU.mult)

            def mm1(c, g):
                logits = psL.tile([128, CH], f32, tag="logits")
                for h in range(CH // 512):
                    nc.tensor.matmul(
                        logits[:, 512 * h:512 * (h + 1)],
                        coefT[:, 128 * g:128 * (g + 1)],
                        basis[c][:, 512 * h:512 * (h + 1)],
                        start=True, stop=True)
                return logits

            sums_t = {}

            def finals(c, s_path=False):
                sums = sums_t.pop(c)
                c0 = c * CH
                r = finpool.tile([D, CH], f32, tag="recip")
                nc.vector.reciprocal_approx_fast(r[:], sums[0:D, :])
                ncorr = finpool.tile([D, CH], f32, tag="ncorr")
                nc.vector.scalar_tensor_tensor(
                    ncorr[:], sums[D:128, :], -1.0, r[:],
                    ALU.mult, ALU.mult, accum_out=egp[:, c:c + 1])
                if s_path:
                    s_path()
                nc.vector.tensor_tensor(Et[:, c0:c0 + CH],
                                        xs[0:D, c0:c0 + CH], ncorr[:],
                                        ALU.add)

            basis_prep(0, halves=True)
            # sum_n x for chunk 0 (others issued inside the loop; all are
            # off the S critical path except none for chunk 3)
            nc.vector.tensor_reduce(egp[:, 4:5], xs[0:D, 0:CH],
                                    mybir.AxisListType.X, ALU.add)

            S = ppool.tile([D, 1], f32, tag="S")
            cc_in = dpool.tile([D, 1], f32, tag="cc_in")
            cc_out2 = dpool.tile([D, 1], f32, tag="cc_out2")
            Sf = ppool.tile([D, 1], f32, tag="Sf")

            def s_path():
                nc.vector.tensor_reduce(S[:], egp[:, 0:8],
                                        mybir.AxisListType.X, ALU.add)
                nc.sync.dma_start(cc_in[:], S[:])
                if use_collective:
                    nc.gpsimd.collective_compute(
                        "AllReduce", ALU.add,
                        replica_groups=[[0, 1], [2, 3], [4, 5], [6, 7]],
                        ins=[cc_in.opt()], outs=[cc_out2.opt()])
                    nc.sync.dma_start(Sf[:], cc_out2[:])
                else:
                    nc.sync.dma_start(Sf[:], cc_in[:])

            units = [(c, g) for c in range(NCH) for g in range(NG)]
            logits_t = {units[0]: mm1(*units[0]),
                        units[1]: mm1(*units[1])}
            for i, (c, g) in enumerate(units):
                if g == 0:
                    sums_t[c] = psS.tile([128, CH], f32, tag="sums",
                                         name=f"sums{c}")
                if g == 11 and c + 1 < NCH:
                    basis_prep(c + 1)
                if (c, g) in ((0, 8), (1, 8), (2, 8)):
                    # sum_n x for chunks 1..3 in DVE slack slots
                    cx = {(0, 8): 1, (1, 8): 2, (2, 8): 3}[(c, g)]
                    nc.vector.tensor_reduce(
                        egp[:, 4 + cx:5 + cx],
                        xs[0:D, cx * CH:(cx + 1) * CH],
                        mybir.AxisListType.X, ALU.add)
                if i + 2 < len(units):
                    logits_t[units[i + 2]] = mm1(*units[i + 2])
                et = epool.tile([128, CH], f16, tag="et")
                L = logits_t.pop((c, g))
                if g in DVEG:
                    nc.vector.tensor_scalar(et[:].bitcast(u16), L[:],
                                            A16, cbx[:, NG + g:NG + g + 1],
                                            ALU.mult, ALU.add)
                else:
                    nc.scalar.activation(et[:], L[:], AF.Exp,
                                         bias=cbx[:, g:g + 1], scale=1.0)
                for h in range(CH // 512):
                    nc.tensor.matmul(
                        sums_t[c][:, 512 * h:512 * (h + 1)],
                        selT[:, 64 + 128 * g:64 + 128 * (g + 1)],
                        et[:, 512 * h:512 * (h + 1)],
                        start=(g == 0), stop=(g == NG - 1),
                        skip_group_check=True)
                if g == NG - 1:
                    finals(c, s_path=s_path if c == NCH - 1 else None)

            # ---- tail: gamma = sigmoid(Sf @ fcwT + fcb) ----
            Sf16 = ppool.tile([D, 1], f16, tag="Sf16")
            nc.vector.tensor_copy(Sf16[:], Sf[:])
            gz = psS.tile([D, 1], f32, tag="sums", name="gz")
            nc.tensor.matmul(gz[:], fcwT[:], Sf16[:], start=True, stop=True)
            ue = ppool.tile([D, 1], f32, tag="ue")
            nc.scalar.activation(ue[:], gz[:], AF.Exp, bias=nfcb[:, 0:1],
                                 scale=-1.0)
            w1 = ppool.tile([D, 1], f32, tag="w1")
            nc.vector.tensor_scalar_add(w1[:], ue[:], 1.0)
            sg = ppool.tile([D, 1], f32, tag="sg")
            nc.vector.reciprocal(sg[:], w1[:])
            g1 = ppool.tile([D, 1], f32, tag="g1")
            nc.vector.tensor_scalar_add(g1[:], sg[:], 1.0)

            # final out = relu(E * (1+gamma)): DVE 2x1280 cols, ACT 1536,
            # each piece DMA'd on its own queue as soon as it's computed
            outt = ppool.tile([D, NL], f32, tag="outt")
            nc.vector.tensor_scalar(outt[:, 0:1280], Et[:, 0:1280],
                                    g1[:, 0:1], 0.0, ALU.mult, ALU.max)
            nc.sync.dma_start(out_d[:, 0:640], outt[:, 0:640])
            nc.gpsimd.dma_start(out_d[:, 640:1280], outt[:, 640:1280])
            nc.vector.tensor_scalar(outt[:, 1280:2560], Et[:, 1280:2560],
                                    g1[:, 0:1], 0.0, ALU.mult, ALU.max)
            nc.sync.dma_start(out_d[:, 1280:1920], outt[:, 1280:1920])
            nc.gpsimd.dma_start(out_d[:, 1920:2560], outt[:, 1920:2560])
            nc.scalar.activation(outt[:, 2560:NL], Et[:, 2560:NL], AF.Relu,
                                 scale=g1[:, 0:1])
            nc.scalar.dma_start(out_d[:, 2560:3328], outt[:, 2560:3328])
            nc.sync.dma_start(out_d[:, 3328:NL], outt[:, 3328:NL])

    nc.compile()
    return nc


def _round8_up(v):
    return np.ceil(np.asarray(v) * 8.0) / 8.0


def _prep_inputs(X, codewords, scale, fc_w, fc_b):
    X = np.ascontiguousarray(np.asarray(X, np.float32))
    cw = np.asarray(codewords, np.float64)
    sc = np.asarray(scale, np.float64)

    a_hi = sc.astype(np.float32).astype(np.float16)
    b_hi = (-2.0 * sc * cw).astype(np.float32).astype(np.float16)
    cterm = (sc * cw * cw).astype(np.float32)
    cw_h = cw.astype(np.float32).astype(np.float16)

    # per-channel softmax-invariant shift keeping e in fp16 range
    smin = np.maximum(-sc.max(axis=0), 0.0)
    t_d = np.minimum(10.0, _round8_up(30.0 * smin)).astype(np.float32)

    coefT = np.zeros((128, 128 * NG), np.float16)
    selTx = np.zeros((128, 64 + 128 * NG), np.float16)
    cbias = np.zeros((128, NG), np.float32)
    for g in range(NG):
        for di in range(4):
            d = 4 * g + di
            m = 128 * g + 32 * di + np.arange(K)
            coefT[d, m] = b_hi[:, d]
            coefT[64 + d, m] = a_hi[:, d]
            cbias[32 * di + np.arange(K), g] = cterm[:, d] + t_d[d]
            selTx[32 * di + np.arange(K), 64 + 128 * g + d] = 1.0
            selTx[32 * di + np.arange(K), 64 + 128 * g + 64 + d] = cw_h[:, d]
    cbias2 = (A16 * cbias + B16).astype(np.float32)
    cbx = np.concatenate([cbias, cbias2], axis=1).astype(np.float32)

    fcwT = np.ascontiguousarray(
        (np.asarray(fc_w, np.float64).T / K).astype(np.float32)).astype(np.float16)
    nfcb = (-np.asarray(fc_b, np.float64)).astype(np.float32).reshape(D, 1)

    Xf = X.reshape(B, D, N)
    in_maps = []
    for core in range(NCORES):
        b, h = core // 2, core % 2
        in_maps.append({
            "x": np.ascontiguousarray(Xf[b, :, h * NL:(h + 1) * NL]),
            "coefT": coefT,
            "selT": selTx,
            "cbx": cbx,
            "fcwT": fcwT,
            "nfcb": nfcb,
        })
    return in_maps


_NC = None


def _get_nc():
    global _NC
    if _NC is None:
        _NC = _build_nc()
    return _NC


def run_sharded(X, codewords, scale, fc_w, fc_b, **spmd_kwargs):
    """Build+run; returns (full_output, BassKernelResults)."""
    nc = _get_nc()
    in_maps = _prep_inputs(X, codewords, scale, fc_w, fc_b)
    res = run_bass_kernel_spmd(nc, in_maps, core_ids=list(range(NCORES)),
                               **spmd_kwargs)
    Y = np.empty((B, D, N), np.float32)
    for core in range(NCORES):
        b, h = core // 2, core % 2
        Y[b, :, h * NL:(h + 1) * NL] = res.results[core]["out"]
    return Y.reshape(B, D, T, H, W), res


def kernel(X, codewords, scale, fc_w, fc_b):
    Y, _ = run_sharded(X, codewords, scale, fc_w, fc_b)
    return Y
